# revision 51
# baseline (speedup 1.0000x reference)
"""Bass/Trainium2 kernel for CausalSelfAttention (B=8, T=1024, C=768, H=12).

Sharding: data-parallel over batch. 8 cores, one batch element per core.
No collectives. Each core runs an identical SPMD program on its own slice.

Schedule: QK/V projection chains are interleaved as PE filler inside the
attention kt-loops so the tensor engine never idles (keeps the PE DVFS
p-state at max clock). PV matmuls lag the S matmuls by one kt step so the
Exp on the scalar engine is off the PE critical path.

Per-core layouts (host-prepared, partition-major so DMA lines are long
and contiguous):
  xT   [128, 2, 6, 512] bf16  x[b].T as (partition, token-half, k-tile, tok)
  wqk  [128, 12, 6, 128] bf16 W_attn[:, :1536] as (p, m-tile, k-tile, col),
                              Q columns pre-scaled by 1/sqrt(64)
  wv   [128, 6, 768] bf16     W_attn[:, 1536:]
  wp   [128, 6, 768] bf16     W_proj
  bqk  [128, 12]  f32     b_attn[:1536] per-tile columns (Q part pre-scaled)
  bv1  [1, 768]   f32     b_attn[1536:]  (broadcast on-chip)
  bp1  [1, 768]   f32     b_proj         (broadcast on-chip)
  qm   [128, 8]   f32     query_mask as per-partition columns per q-tile
  dm   [128, 8, 128] bf16 diagonal-block multiplicative masks, transposed
Output: y [1024, 768] f32 per core.
"""

import sys

if "/opt/trn_rl_repo" not in sys.path:
    sys.path.insert(0, "/opt/trn_rl_repo")

import numpy as np
import ml_dtypes

import concourse.bass as bass
import concourse.bacc as bacc
import concourse.mybir as mybir
import concourse.tile as tile
from concourse.bass import ts, ds

BF16 = mybir.dt.bfloat16
F32 = mybir.dt.float32
AF = mybir.ActivationFunctionType
ALU = mybir.AluOpType
BF16NP = ml_dtypes.bfloat16

T, C, H, HD = 1024, 768, 12, 64
NCORES = 8

_CACHE = {}


def build_program():
    """Build the single-core SPMD Bass program."""
    nc = bacc.Bacc("TRN2", target_bir_lowering=False, debug=False)

    xT_d = nc.dram_tensor("xT", [128, 2, 6, 512], BF16, kind="ExternalInput")
    wqk_d = nc.dram_tensor("wqk", [128, 12, 6, 128], BF16, kind="ExternalInput")
    wv_d = nc.dram_tensor("wv", [128, 6, C], BF16, kind="ExternalInput")
    wp_d = nc.dram_tensor("wp", [128, 6, C], BF16, kind="ExternalInput")
    bqk_d = nc.dram_tensor("bqk", [128, 12], F32, kind="ExternalInput")
    bv1_d = nc.dram_tensor("bv1", [1, C], F32, kind="ExternalInput")
    bp1_d = nc.dram_tensor("bp1", [1, C], F32, kind="ExternalInput")
    qm_d = nc.dram_tensor("qm", [128, 8], F32, kind="ExternalInput")
    dm_d = nc.dram_tensor("dm", [128, 8, 128], BF16, kind="ExternalInput")
    y_d = nc.dram_tensor("y", [T, C], F32, kind="ExternalOutput")

    with tile.TileContext(nc) as tc:
        with (
            tc.tile_pool(name="const", bufs=1) as cp,
            tc.tile_pool(name="ptp", bufs=12) as ptp,
            tc.tile_pool(name="recp", bufs=3) as recp,
            tc.tile_pool(name="bcp", bufs=3) as bcp,
            tc.tile_pool(name="otxp", bufs=3) as otxp,
            tc.tile_pool(name="ysb", bufs=3) as ysbp,
            tc.tile_pool(name="ps_a", bufs=5, space="PSUM") as ps_a,
            tc.tile_pool(name="ps_o", bufs=2, space="PSUM") as ps_o,
            tc.tile_pool(name="ps_bc", bufs=1, space="PSUM") as ps_bc,
        ):
            # ---------------- persistent SBUF tensors ----------------
            xT_sb = cp.tile([128, 2, 6, 512], BF16, name="xT_sb")
            wqk_sb = cp.tile([128, 12, 6, 128], BF16, name="wqk_sb")
            wv_sb = cp.tile([128, 6, C], BF16, name="wv_sb")
            wp_sb = cp.tile([128, 6, C], BF16, name="wp_sb")
            bqk_sb = cp.tile([128, 12], F32, name="bqk_sb")
            bv_sb = cp.tile([128, C], F32, name="bv_sb")
            bp_sb = cp.tile([128, C], F32, name="bp_sb")
            bv1_sb = cp.tile([1, C], F32, name="bv1_sb")
            bp1_sb = cp.tile([1, C], F32, name="bp1_sb")
            qm_sb = cp.tile([128, 8], F32, name="qm_sb")
            dm_sb = cp.tile([128, 8, 128], BF16, name="dm_sb")
            qk_sb = [cp.tile([128, T], BF16, name=f"qk{m}") for m in range(12)]
            v_sb = [cp.tile([128, 12 * 65], BF16, name=f"v{t}") for t in range(8)]
            ot_sb = cp.tile([128, 6, T], BF16, name="ot_sb")

            # ---------------- loads (priority ordered) ----------------
            # Each dma_start spreads over only a few DMA queues, so large
            # tensors are split across several issues on the three DMA-
            # capable engine queues (sync/scalar/gpsimd); issue order gives
            # the priority stagger. Flat 2D APs (contiguous per partition on
            # both sides) keep descriptor generation fast.
            xTf_d = xT_d[:, :, :, :].rearrange("p a b c -> p (a b c)")
            xTf_s = xT_sb[:, :, :, :].rearrange("p a b c -> p (a b c)")
            wqf_d = wqk_d[:, :, :, :].rearrange("p a b c -> p (a b c)")
            wqf_s = wqk_sb[:, :, :, :].rearrange("p a b c -> p (a b c)")
            wvf_d = wv_d[:, :, :].rearrange("p a b -> p (a b)")
            wvf_s = wv_sb[:, :, :].rearrange("p a b -> p (a b)")
            wpf_d = wp_d[:, :, :].rearrange("p a b -> p (a b)")
            wpf_s = wp_sb[:, :, :].rearrange("p a b -> p (a b)")
            dmf_d = dm_d[:, :, :].rearrange("p a b -> p (a b)")
            dmf_s = dm_sb[:, :, :].rearrange("p a b -> p (a b)")

            # -- critical set: prelude (qk m0/m6 on tokens 0:512) + pr0 --
            nc.sync.dma_start(bqk_sb[:], bqk_d[:, :])
            nc.scalar.dma_start(bv1_sb[:], bv1_d[:, :])
            nc.scalar.dma_start(bp1_sb[:], bp1_d[:, :])
            nc.sync.dma_start(xTf_s[:, 0:1536], xTf_d[:, 0:1536])
            nc.scalar.dma_start(xTf_s[:, 1536:3072], xTf_d[:, 1536:3072])
            nc.sync.dma_start(wqf_s[:, 0:768], wqf_d[:, 0:768])
            nc.gpsimd.dma_start(wqf_s[:, 4608:5376], wqf_d[:, 4608:5376])
            nc.gpsimd.dma_start(dmf_s[:, :], dmf_d[:, :])
            nc.gpsimd.dma_start(wvf_s[:, 0:2304], wvf_d[:, 0:2304])
            nc.gpsimd.dma_start(wvf_s[:, 2304:4608], wvf_d[:, 2304:4608])
            # -- mid set: second token half + qk m-tiles for pr1 --
            nc.sync.dma_start(xTf_s[:, 3072:4608], xTf_d[:, 3072:4608])
            nc.scalar.dma_start(xTf_s[:, 4608:6144], xTf_d[:, 4608:6144])
            nc.sync.dma_start(wqf_s[:, 768:2304], wqf_d[:, 768:2304])
            nc.scalar.dma_start(wqf_s[:, 5376:6912], wqf_d[:, 5376:6912])
            # -- late set: remaining qk m-tiles + proj weights --
            nc.sync.dma_start(wqf_s[:, 2304:4608], wqf_d[:, 2304:4608])
            nc.scalar.dma_start(wqf_s[:, 6912:9216], wqf_d[:, 6912:9216])
            nc.sync.dma_start(wpf_s[:, 0:2304], wpf_d[:, 0:2304])
            nc.scalar.dma_start(wpf_s[:, 2304:4608], wpf_d[:, 2304:4608])
            nc.scalar.dma_start(qm_sb[:], qm_d[:, :])
            # warm the Exp activation table during the DMA ramp
            wrm = cp.tile([1, 16], F32, name="wrm")
            wrm2 = cp.tile([1, 16], F32, name="wrm2")
            nc.gpsimd.memset(wrm[:], 0.0)
            nc.scalar.activation(wrm2[:], wrm[:], AF.Exp)
            # ones columns interleaved into V (softmax sums via PV row 64)
            ones_bf = cp.tile([128, 64], BF16, name="ones_bf")
            nc.gpsimd.memset(ones_bf[:], 1.0)
            for t in range(8):
                nc.gpsimd.memset(
                    v_sb[t].rearrange("p (h d) -> p h d", d=65)[:, :, 64:65], 1.0
                )
            # bias broadcasts run as attention fills so they queue on
            # gpsimd BEHIND the first mask-muls, not ahead of them
            def bcast_bv():
                nc.gpsimd.partition_broadcast(bv_sb[:], bv1_sb[:])

            def bcast_bp():
                nc.gpsimd.partition_broadcast(bp_sb[:], bp1_sb[:])

            # ---------------- projection chains (used inline + as filler) ----------------
            def qk_chain(m, j):
                ps = ps_a.tile([128, 512], F32, name="ps", tag="a")
                for k in range(6):
                    nc.tensor.matmul(
                        ps[:],
                        wqk_sb[:, m, k, :],
                        xT_sb[:, j, k, :],
                        start=(k == 0),
                        stop=(k == 5),
                    )
                nc.vector.tensor_scalar(
                    qk_sb[m][:, ts(j, 512)],
                    ps[:],
                    bqk_sb[:, m : m + 1],
                    None,
                    op0=ALU.add,
                )

            def v_chain(t, half):
                c0, cw = (0, 512) if half == 0 else (512, 256)
                psv = ps_a.tile([128, 512], F32, name="psv", tag="a")
                for k in range(6):
                    nc.tensor.matmul(
                        psv[:, :cw],
                        xT_sb[:, t // 4, k, ds((t % 4) * 128, 128)],
                        wv_sb[:, k, ds(c0, cw)],
                        start=(k == 0),
                        stop=(k == 5),
                    )
                nh, h0 = cw // 64, c0 // 64
                nc.vector.tensor_add(
                    v_sb[t].rearrange("p (h d) -> p h d", d=65)[
                        :, h0 : h0 + nh, 0:64
                    ],
                    psv[:, :cw].rearrange("p (h d) -> p h d", d=64),
                    bv_sb[:, ds(c0, cw)].rearrange("p (h d) -> p h d", d=64),
                )

            def F(m, j):
                return lambda: qk_chain(m, j)

            def V(t, half):
                return lambda: v_chain(t, half)

            # ---------------- attention group: one head-pair, one query half ----------------
            def attention_group(pr, sbi, fills):
                hs = (2 * pr, 2 * pr + 1)
                q0 = 512 * sbi
                nkt = 4 + 4 * sbi
                psO = {
                    h: ps_o.tile([65, 512], F32, name="op", tag="op") for h in hs
                }
                ptts = {}

                def dopv(kt):
                    dc = max(0, kt * 128 - q0)
                    w = 512 - dc
                    for h in hs:
                        nc.tensor.matmul(
                            psO[h][:, ds(dc, w)],
                            v_sb[kt][:, h * 65 : h * 65 + 65],
                            ptts.pop((h, kt))[:, ds(dc, w)],
                            start=(kt == 0),
                            stop=(kt == nkt - 1),
                            skip_group_check=True,
                        )

                nf = len(fills)
                fi = 0
                for kt in range(nkt):
                    dc = max(0, kt * 128 - q0)
                    w = 512 - dc
                    for h in hs:
                        qp = (h % 2) * 64
                        sp = ps_a.tile([128, 512], F32, name="sp", tag="a")
                        nc.tensor.matmul(
                            sp[:, ds(dc, w)],
                            qk_sb[6 + pr][qp : qp + 64, ts(kt, 128)],
                            qk_sb[pr][qp : qp + 64, ds(q0 + dc, w)],
                            start=True,
                            stop=True,
                        )
                        ptt = ptp.tile([128, 512], BF16, name="ptt", tag="ptt")
                        nc.scalar.activation(
                            ptt[:, ds(dc, w)], sp[:, ds(dc, w)], AF.Exp
                        )
                        if kt * 128 >= q0:
                            nc.gpsimd.tensor_mul(
                                ptt[:, ds(dc, 128)],
                                ptt[:, ds(dc, 128)],
                                dm_sb[:, kt, :],
                            )
                        ptts[(h, kt)] = ptt
                    # pace the filler chains evenly across kt steps
                    tgt = ((kt + 1) * nf + nkt - 1) // nkt
                    while fi < tgt:
                        fills[fi]()
                        fi += 1
                    if kt >= 1:
                        dopv(kt - 1)
                dopv(nkt - 1)

                # normalize: OT = psO[0:64] / sum  (sum = psO row 64)
                # sums -> bf16 sbuf -> PE broadcast [64,512] -> approx recip
                for h in hs:
                    sums = recp.tile([65, 512], BF16, name="sums", tag="sums")
                    nc.vector.tensor_copy(sums[64:65, :], psO[h][64:65, :])
                    bc = ps_bc.tile([64, 512], F32, name="bc", tag="bc")
                    nc.tensor.matmul(
                        bc[:],
                        ones_bf[64:65, 0:64],
                        sums[64:65, :],
                        start=True,
                        stop=True,
                    )
                    bcs = bcp.tile([64, 512], F32, name="bcs", tag="bcs")
                    nc.vector.reciprocal_approx_fast(bcs[:], bc[:])
                    if h % 2 == 0:
                        nc.vector.tensor_mul(
                            ot_sb[0:64, pr, ds(q0, 512)],
                            psO[h][0:64, :],
                            bcs[:],
                        )
                    else:
                        otx = otxp.tile([64, 512], BF16, name="otx", tag="otx")
                        nc.vector.tensor_mul(otx[:], psO[h][0:64, :], bcs[:])
                        nc.sync.dma_start(
                            ot_sb[64:128, pr, ds(q0, 512)], otx[:]
                        )

            # ---------------- main schedule ----------------
            qk_chain(0, 0)
            qk_chain(6, 0)
            attention_group(
                0, 0,
                [bcast_bv, V(0, 0), V(0, 1), V(1, 0), V(1, 1),
                 V(2, 0), V(2, 1), V(3, 0), V(3, 1)],
            )
            qk_chain(1, 0)
            qk_chain(0, 1)
            attention_group(
                0, 1,
                [F(6, 1), V(4, 0), V(4, 1), F(7, 0), V(5, 0),
                 V(5, 1), V(6, 0), V(6, 1), V(7, 0), V(7, 1),
                 F(1, 1), F(7, 1)],
            )
            for pr in (1, 2, 3):
                attention_group(
                    pr, 0,
                    ([bcast_bp] if pr == 1 else []) + [F(pr + 1, 0), F(pr + 7, 0)],
                )
                attention_group(pr, 1, [F(pr + 1, 1), F(pr + 7, 1)])
            attention_group(4, 0, [F(5, 0), F(11, 0)])
            attention_group(4, 1, [F(5, 1)])
            attention_group(5, 0, [F(11, 1)])
            attention_group(5, 1, [])

            # ---------------- phase E: y = OT.T @ W_proj * qm + bp ----------------
            for qt in range(8):
                ysb = ysbp.tile([128, C], F32, name="ysb", tag="ysb")
                for c0, cw in ((0, 512), (512, 256)):
                    psy = ps_a.tile([128, 512], F32, name="psy", tag="a")
                    for k in range(6):
                        nc.tensor.matmul(
                            psy[:, :cw],
                            ot_sb[:, k, ts(qt, 128)],
                            wp_sb[:, k, ds(c0, cw)],
                            start=(k == 0),
                            stop=(k == 5),
                        )
                    nc.vector.scalar_tensor_tensor(
                        out=ysb[:, ds(c0, cw)],
                        in0=psy[:, :cw],
                        scalar=qm_sb[:, qt : qt + 1],
                        in1=bp_sb[:, ds(c0, cw)],
                        op0=ALU.mult,
                        op1=ALU.add,
                    )
                nc.sync.dma_start(y_d[ts(qt, 128), :], ysb[:])

    nc.compile()
    return nc


def _get_nc():
    if "nc" not in _CACHE:
        _CACHE["nc"] = build_program()
    return _CACHE["nc"]


def prep_core_inputs(x, mask, query_mask, W_attn, b_attn, W_proj, b_proj):
    """Host-side prep. Returns a list of per-core input dicts."""
    scale = 1.0 / np.sqrt(HD)
    W_s = np.asarray(W_attn, np.float32).copy()
    W_s[:, :C] *= scale
    b_s = np.asarray(b_attn, np.float32).copy()
    b_s[:C] *= scale

    def shuf(w):
        # [768, X] -> [128, 6, X]: partition-major layout matching SBUF tiles
        w = np.asarray(w)
        return np.ascontiguousarray(
            w.reshape(6, 128, w.shape[1]).transpose(1, 0, 2)
        )

    def shuf4(w, n, width):
        # [768, n*width] -> [128, n, 6, width]: m-tile-major so per-m loads
        # are contiguous per partition
        w = np.asarray(w)
        return np.ascontiguousarray(
            w.reshape(6, 128, n, width).transpose(1, 2, 0, 3)
        )

    shared = {
        "wqk": shuf4(W_s[:, : 2 * C].astype(BF16NP), 12, 128),
        "wv": shuf(W_s[:, 2 * C :].astype(BF16NP)),
        "wp": shuf(np.asarray(W_proj, np.float32).astype(BF16NP)),
        "bqk": np.ascontiguousarray(b_s[: 2 * C].reshape(12, 128).T),
        "bv1": np.ascontiguousarray(b_s[2 * C :].reshape(1, C)).astype(
            np.float32
        ),
        "bp1": np.ascontiguousarray(
            np.asarray(b_proj, np.float32).reshape(1, C)
        ),
    }

    per_core = []
    for b in range(NCORES):
        xT = shuf4(np.asarray(x[b], np.float32).T.astype(BF16NP), 2, 512)
        qm = np.ascontiguousarray(
            np.asarray(query_mask[b, 0, :, 0], np.float32).reshape(8, 128).T
        )
        mb = np.asarray(mask[b, 0])  # [T, T] bool
        blocks = [
            mb[qi * 128 : (qi + 1) * 128, qi * 128 : (qi + 1) * 128].T
            for qi in range(8)
        ]
        dm = np.stack(blocks, axis=1).astype(BF16NP)  # [128, 8, 128]
        per_core.append({"xT": xT, "qm": qm, "dm": dm, **shared})
    return per_core


def run_on_cores(inputs, trace=False, **kw):
    from concourse.bass_utils import run_bass_kernel_spmd

    nc = _get_nc()
    in_maps = prep_core_inputs(**inputs)
    res = run_bass_kernel_spmd(
        nc, in_maps, core_ids=list(range(NCORES)), trace=trace, **kw
    )
    out = np.stack([res.results[b]["y"] for b in range(NCORES)], axis=0)
    return out.astype(np.float32), res


def kernel(**inputs) -> np.ndarray:
    out, _ = run_on_cores(inputs, trace=False)
    return out
